# revision 36
# baseline (speedup 1.0000x reference)
"""ALiBi causal attention layer on 8 TRN2 NeuronCores.

Sharding: data parallel on batch (B=2) x tensor parallel on heads (16 -> 4
groups of 4).  Core c = 4*b + g computes, for batch element b, the 4 heads
[4g, 4g+4) end to end: QKV projections (column-sharded), causal ALiBi
attention, and the row-sharded output projection.  The host sums the 4
partial outputs per batch element (the tensor-parallel all-reduce) and adds
the output bias.

Device kernel (all matmuls in float32r, ~1e-4 rel err, fp32 PSUM accum):
  - x arrives host-transposed with a ones row: xt [1025, 2048]; projection
    biases ride in an augmented contraction row of each weight matrix.
  - K^T lives in per-head [128, 2048] tiles: head data at its native
    partition parity (even head rows 0:64, odd rows 64:128), the ALiBi
    rank-2 rows (slope*8*k, ones) adjacent, remaining rows zeroed.  Q^T
    uses matching per-(head, q-block) [128, 512] tiles with rows
    (ones, -slope*8*q).  S^T = K_aug^T.T @ Q_aug then exp() directly on
    ACT with scale=1/8 (max-free softmax: scores are bounded), so S^T
    already includes the ALiBi bias.
  - Causality: k-tiles fully above the diagonal are skipped; diagonal
    tiles are zero-filled post-exp with gpsimd.affine_select.
  - V carries a ones column per head, so the PV matmul yields O^T plus the
    softmax denominators; O^T *= 1/den via DVE reciprocal + PE broadcast.
"""
import math

import numpy as np

import concourse.bass as bass
import concourse.tile as tile
from concourse import mybir, bacc
from concourse.bass_utils import run_bass_kernel_spmd

F32 = mybir.dt.float32
F32R = mybir.dt.float32r

B, T, C, H = 2, 2048, 1024, 16
D = C // H            # 64 head dim
NCORES = 8
HG = 4                # heads per core
CG = HG * D           # 256 channels per core
VW = HG * (D + 1)     # 260: V with a ones column per head
QB = 512              # q block width
KTW = 128             # k tile width
NQB = T // QB         # 4
NKT = T // KTW        # 16
NCH = C // 128        # 8 contraction chunks


def _slopes(n):
    def p2(m):
        start = 2 ** (-(2 ** -(math.log2(m) - 3)))
        return [start * start**i for i in range(m)]
    if math.log2(n).is_integer():
        return p2(n)
    c = 2 ** math.floor(math.log2(n))
    return p2(c) + _slopes(2 * c)[0::2][: n - c]


def _build():
    nc = bacc.Bacc()
    xt = nc.declare_dram_parameter("xt", [C + 1, T], F32R, isOutput=False)
    wq = nc.declare_dram_parameter("wq", [C + 1, CG], F32R, isOutput=False)
    wk = nc.declare_dram_parameter("wk", [C + 1, CG], F32R, isOutput=False)
    wv = nc.declare_dram_parameter("wv", [C + 1, VW], F32R, isOutput=False)
    wo = nc.declare_dram_parameter("wo", [CG, C], F32R, isOutput=False)
    hka = nc.declare_dram_parameter("hka", [HG, 2, T], F32R, isOutput=False)
    hqa = nc.declare_dram_parameter("hqa", [HG, 2, T], F32R, isOutput=False)
    y = nc.declare_dram_parameter("y", [T, C], F32, isOutput=True)

    EXP = mybir.ActivationFunctionType.Exp

    with tile.TileContext(nc) as tc, \
         nc.allow_low_precision(reason="fp32r compute"):
        with tc.tile_pool(name="const", bufs=1) as cp, \
             tc.tile_pool(name="xtp", bufs=14) as xtp, \
             tc.tile_pool(name="qap", bufs=8) as qap, \
             tc.tile_pool(name="otp", bufs=4) as otp, \
             tc.tile_pool(name="ptp", bufs=4) as ptp, \
             tc.tile_pool(name="yp", bufs=2) as ypool, \
             tc.tile_pool(name="misc", bufs=2) as mp, \
             tc.tile_pool(name="ps", bufs=2, space="PSUM") as psp, \
             tc.tile_pool(name="psn", bufs=2, space="PSUM") as psn, \
             tc.tile_pool(name="po", bufs=2, space="PSUM") as pop:

            # ---- constants: weights, aug rows, zero fill ----
            # DMA emission order matters for time-to-first-matmul: wq and
            # the first x block go first so the Q projection can start while
            # the rest of the constants stream in.
            wq_sb = [cp.tile([128, CG], F32R, tag=f"wq{c}", name=f"wq{c}") for c in range(NCH)]
            wk_sb = [cp.tile([128, CG], F32R, tag=f"wk{c}", name=f"wk{c}") for c in range(NCH)]
            wv_sb = [cp.tile([128, VW], F32R, tag=f"wv{c}", name=f"wv{c}") for c in range(NCH)]
            wo_sb = [cp.tile([128, C], F32R, tag=f"wo{c}", name=f"wo{c}") for c in range(2)]
            wqb = cp.tile([1, CG], F32R, tag="wqb")
            wkb = cp.tile([1, CG], F32R, tag="wkb")
            wvb = cp.tile([1, VW], F32R, tag="wvb")
            ones_sb = cp.tile([1, QB], F32R, tag="ones")
            xts0 = []
            for c in range(NCH):
                nc.sync.dma_start(wq_sb[c][:], wq[128 * c:128 * (c + 1), :])
                xtt = xtp.tile([128, QB], F32R, tag="xt", name=f"xt0_{c}")
                nc.sync.dma_start(xtt[:], xt[128 * c:128 * (c + 1), 0:QB])
                xts0.append(xtt)
            nc.sync.dma_start(wqb[:], wq[C:C + 1, :])
            nc.sync.dma_start(ones_sb[:], xt[C:C + 1, 0:QB])

            for c in range(NCH):
                nc.sync.dma_start(wk_sb[c][:], wk[128 * c:128 * (c + 1), :])
            nc.sync.dma_start(wkb[:], wk[C:C + 1, :])

            zf = cp.tile([128, QB], F32, tag="zf")
            nc.vector.memset(zf[:], 0.0)

            for c in range(NCH):
                nc.sync.dma_start(wv_sb[c][:], wv[128 * c:128 * (c + 1), :])
            nc.sync.dma_start(wvb[:], wv[C:C + 1, :])
            for c in range(2):
                nc.sync.dma_start(wo_sb[c][:], wo[128 * c:128 * (c + 1), :])

            # Per-head K^T tiles (resident, full T).  Even head: data rows
            # 0:64, aug rows 64:66, zeros 66:128.  Odd head: aug rows 0:2,
            # zeros 2:64, data rows 64:128.  K aug = (slope8*k, ones).
            ka = [cp.tile([128, T], F32R, tag=f"ka{h}", name=f"ka{h}") for h in range(HG)]
            for h in range(HG):
                par = h % 2
                arow = 64 if par == 0 else 0
                # zero the whole non-data half (32-aligned partition base),
                # then the aug-row DMA overwrites its 2 rows
                for blk in range(NQB):
                    sl = slice(QB * blk, QB * (blk + 1))
                    nc.vector.tensor_copy(ka[h][arow:arow + 64, sl],
                                          zf[arow:arow + 64, :])
                nc.sync.dma_start(ka[h][arow:arow + 2, :], hka[h])

            v_sb = [cp.tile([128, VW], F32R, tag=f"v{t}", name=f"v{t}") for t in range(NKT)]

            # ---- fused, software-pipelined per-block loop ----
            def proj(qb):
                """QKV projections for t-block qb; returns the Q tiles."""
                tsl = slice(QB * qb, QB * (qb + 1))
                if qb == 0:
                    xts = xts0
                else:
                    xts = []
                    for c in range(NCH):
                        xtt = xtp.tile([128, QB], F32R, tag="xt",
                                       name=f"xt{qb}_{c}")
                        nc.sync.dma_start(xtt[:],
                                          xt[128 * c:128 * (c + 1), tsl])
                        xts.append(xtt)

                qa_t = []
                for h in range(HG):
                    qat = qap.tile([128, QB], F32R, tag="qa",
                                   name=f"qa{qb}_{h}")
                    par = h % 2
                    arow = 64 if par == 0 else 0
                    nc.vector.tensor_copy(qat[arow:arow + 64, :],
                                          zf[arow:arow + 64, :])
                    nc.sync.dma_start(qat[arow:arow + 2, :], hqa[h][:, tsl])
                    qa_t.append(qat)

                for wsb, wb, is_q in ((wq_sb, wqb, True), (wk_sb, wkb, False)):
                    for m in range(2):
                        ps = psn.tile([128, QB], F32, tag="psn")
                        for c in range(NCH):
                            nc.tensor.matmul(
                                ps[:], wsb[c][:, 128 * m:128 * (m + 1)],
                                xts[c][:], start=(c == 0), stop=False,
                                skip_group_check=True)
                        nc.tensor.matmul(
                            ps[:], wb[:, 128 * m:128 * (m + 1)], ones_sb[:],
                            start=False, stop=True, skip_group_check=True)
                        for j in range(2):
                            h = 2 * m + j
                            rows = slice(64 * j, 64 * j + 64)
                            if is_q:
                                nc.vector.tensor_copy(qa_t[h][rows, :],
                                                      ps[rows, :])
                            else:
                                nc.vector.tensor_copy(ka[h][rows, tsl],
                                                      ps[rows, :])

                for tt in range(4):
                    kt = 4 * qb + tt
                    psv = psn.tile([128, QB], F32, tag="psn")
                    for c in range(NCH):
                        nc.tensor.matmul(
                            psv[:, 0:VW],
                            xts[c][:, 128 * tt:128 * (tt + 1)], wv_sb[c][:],
                            start=(c == 0), stop=False, skip_group_check=True)
                    nc.tensor.matmul(
                        psv[:, 0:VW], ones_sb[:, 0:128], wvb[:],
                        start=False, stop=True, skip_group_check=True)
                    nc.vector.tensor_copy(v_sb[kt][:], psv[:, 0:VW])
                return qa_t

            for qb in range(NQB):
                qa_t = proj(qb)
                # attention for this q-block.  Pass A per head is the
                # PE-heavy S/exp/mask/PV chain; pass B (recip -> broadcast
                # -> divide) for head h is emitted after head h+1's pass A
                # so the broadcast matmul never sits at the front of the PE
                # queue waiting on the DVE reciprocal.
                po_t = {}
                ot_t = [otp.tile([128, QB], F32R, tag="ot",
                                 name=f"ot_{qb}_{c}") for c in range(2)]

                def pass_a(h):
                    # adjacent k tiles are paired side by side in one 2-bank
                    # PSUM tile so a single ACTIVATE covers both: the
                    # attention stretch is exp-rate-bound on ACT, and the
                    # ~350-cycle per-call pipeline fill was ~1/3 of it
                    nkt = 4 * qb + 4
                    po = pop.tile([D + 1, QB], F32, tag="po",
                                  name=f"po_{qb}_{h}")
                    for p in range(0, nkt, 2):
                        pss = psp.tile([128, 2 * QB], F32, tag="ps",
                                       name=f"pss_{qb}_{h}_{p}")
                        for j in range(2):
                            kt = p + j
                            nc.tensor.matmul(
                                pss[:, QB * j:QB * (j + 1)],
                                ka[h][:, 128 * kt:128 * (kt + 1)],
                                qa_t[h][:], start=True, stop=True,
                                skip_group_check=True)
                        pt = ptp.tile([128, 2 * QB], F32R, tag="pt")
                        nc.scalar.activation(pt[:], pss[:], EXP,
                                             bias=0.0, scale=0.125)
                        for j in range(2):
                            kt = p + j
                            d0 = 128 * kt - QB * qb
                            if d0 >= 0:
                                # keep where k <= q, i.e. f - p - d0 >= 0
                                nc.gpsimd.affine_select(
                                    pt[:, QB * j:QB * (j + 1)],
                                    pt[:, QB * j:QB * (j + 1)],
                                    pattern=[[1, QB]], base=-d0,
                                    channel_multiplier=-1,
                                    compare_op=mybir.AluOpType.is_ge,
                                    fill=0.0)
                        for j in range(2):
                            kt = p + j
                            nc.tensor.matmul(
                                po[:], v_sb[kt][:, 65 * h:65 * (h + 1)],
                                pt[:, QB * j:QB * (j + 1)],
                                start=(kt == 0), stop=(kt == nkt - 1),
                                skip_group_check=True)
                    den = mp.tile([1, QB], F32, tag="den", bufs=2,
                                  name=f"den_{qb}_{h}")
                    nc.vector.tensor_copy(den[:], po[D:D + 1, :])
                    rc32 = mp.tile([1, QB], F32, tag="rc32", bufs=2,
                                   name=f"rc32_{qb}_{h}")
                    nc.vector.reciprocal_approx_fast(rc32[:], den[:])
                    rc = mp.tile([1, QB], F32R, tag="rc", bufs=4,
                                 name=f"rc_{qb}_{h}")
                    nc.vector.tensor_copy(rc[:], rc32[:])
                    po_t[h] = (po, rc)

                def pass_b(h):
                    po, rc = po_t.pop(h)
                    pb = psn.tile([D, QB], F32, tag="psn",
                                  name=f"pb_{qb}_{h}")
                    nc.tensor.matmul(pb[:], ones_sb[:, 0:D], rc[:],
                                     start=True, stop=True,
                                     skip_group_check=True)
                    bc = mp.tile([D, QB], F32, tag="bc", bufs=4,
                                 name=f"bc_{qb}_{h}")
                    nc.vector.tensor_copy(bc[:], pb[:])
                    pair = ot_t[h // 2]
                    if h % 2 == 0:
                        nc.vector.tensor_tensor(pair[0:D, :], po[0:D, :],
                                                bc[:],
                                                op=mybir.AluOpType.mult)
                    else:
                        # odd head's O^T lands at partitions 0:64; DVE
                        # cannot shift partitions, so divide into a temp
                        # then DMA it into rows 64:128 of the pair tile
                        tmp = mp.tile([D, QB], F32R, tag="ottmp", bufs=4,
                                      name=f"ottmp_{qb}_{h}")
                        nc.vector.tensor_tensor(tmp[:], po[0:D, :], bc[:],
                                                op=mybir.AluOpType.mult)
                        nc.sync.dma_start(pair[D:2 * D, :], tmp[:])

                for h in range(HG):
                    pass_a(h)
                    if h >= 1:
                        pass_b(h - 1)
                pass_b(HG - 1)

                # output projection for this t-block
                for tt in range(4):
                    t = 4 * qb + tt
                    fsl = slice(128 * tt, 128 * (tt + 1))
                    ysb = ypool.tile([128, C], F32, tag="y",
                                     name=f"y_{qb}_{tt}")
                    for half in range(2):
                        hsl = slice(QB * half, QB * (half + 1))
                        py = psn.tile([128, QB], F32, tag="psn")
                        for c in range(2):
                            nc.tensor.matmul(
                                py[:], ot_t[c][:, fsl], wo_sb[c][:, hsl],
                                start=(c == 0), stop=(c == 1),
                                skip_group_check=True)
                        nc.vector.tensor_copy(ysb[:, hsl], py[:])
                    nc.sync.dma_start(y[128 * t:128 * (t + 1), :], ysb[:])
    nc.finalize()
    return nc


_NC_CACHE = None


def _get_nc():
    global _NC_CACHE
    if _NC_CACHE is None:
        _NC_CACHE = _build()
    return _NC_CACHE


def kernel(x, Wq, bq, Wk, bk, Wv, bv, Wo, bo):
    x = np.asarray(x, dtype=np.float32)
    Wq, bq = np.asarray(Wq, np.float32), np.asarray(bq, np.float32)
    Wk, bk = np.asarray(Wk, np.float32), np.asarray(bk, np.float32)
    Wv, bv = np.asarray(Wv, np.float32), np.asarray(bv, np.float32)
    Wo, bo = np.asarray(Wo, np.float32), np.asarray(bo, np.float32)

    slopes = np.asarray(_slopes(H), dtype=np.float32)
    ar = np.arange(T, dtype=np.float32)

    xts = []
    for b in range(B):
        xa = np.empty((C + 1, T), np.float32)
        xa[:C] = x[b].T
        xa[C] = 1.0
        xts.append(np.ascontiguousarray(xa))

    shards = []
    for g in range(HG):
        csl = slice(CG * g, CG * (g + 1))
        wqa = np.concatenate([Wq[:, csl], bq[None, csl]], axis=0)
        wka = np.concatenate([Wk[:, csl], bk[None, csl]], axis=0)
        wva = np.zeros((C + 1, VW), np.float32)
        for j in range(HG):
            src = slice(CG * g + D * j, CG * g + D * (j + 1))
            wva[:C, 65 * j:65 * j + D] = Wv[:, src]
            wva[C, 65 * j:65 * j + D] = bv[src]
            wva[C, 65 * j + D] = 1.0
        woa = np.ascontiguousarray(Wo[csl, :])
        hk = np.empty((HG, 2, T), np.float32)
        hq = np.empty((HG, 2, T), np.float32)
        for j in range(HG):
            # K rows (k, s8) pair with Q rows (s8, -q): S += s8*(k - q).
            # Integer k/q are exact on the f32r grid and s8 rounds once, so
            # the large terms cancel exactly in the fp32 PSUM accumulator
            # (splitting s8*k / s8*q would round each entry independently
            # and leave O(s8*T*eps) noise in the scores).
            s8 = 8.0 * slopes[HG * g + j]
            hk[j, 0] = ar
            hk[j, 1] = s8
            hq[j, 0] = s8
            hq[j, 1] = -ar
        shards.append(dict(
            wq=np.ascontiguousarray(wqa), wk=np.ascontiguousarray(wka),
            wv=wva, wo=woa, hka=hk, hqa=hq))

    in_maps = []
    for core in range(NCORES):
        b, g = divmod(core, HG)
        m = dict(shards[g])
        m["xt"] = xts[b]
        in_maps.append(m)

    nc = _get_nc()
    res = run_bass_kernel_spmd(nc, in_maps, core_ids=list(range(NCORES)))

    out = np.empty((B, T, C), np.float32)
    for b in range(B):
        acc = res.results[4 * b]["y"].astype(np.float32).copy()
        for g in range(1, HG):
            acc += res.results[4 * b + g]["y"]
        out[b] = acc + bo[None, :]
    return out


# revision 37
# speedup vs baseline: 1.0890x; 1.0890x over previous
"""ALiBi causal attention layer on 8 TRN2 NeuronCores.

Sharding: data parallel on batch (B=2) x tensor parallel on heads (16 -> 4
groups of 4).  Core c = 4*b + g computes, for batch element b, the 4 heads
[4g, 4g+4) end to end: QKV projections (column-sharded), causal ALiBi
attention, and the row-sharded output projection.  The host sums the 4
partial outputs per batch element (the tensor-parallel all-reduce) and adds
the output bias.

Device kernel (all matmuls in float32r, ~1e-4 rel err, fp32 PSUM accum):
  - x arrives host-transposed with a ones row: xt [1025, 2048]; projection
    biases ride in an augmented contraction row of each weight matrix.
  - K^T lives in per-head [128, 2048] tiles: head data at its native
    partition parity (even head rows 0:64, odd rows 64:128), the ALiBi
    rank-2 rows (slope*8*k, ones) adjacent, remaining rows zeroed.  Q^T
    uses matching per-(head, q-block) [128, 512] tiles with rows
    (ones, -slope*8*q).  S^T = K_aug^T.T @ Q_aug then exp() directly on
    ACT with scale=1/8 (max-free softmax: scores are bounded), so S^T
    already includes the ALiBi bias.
  - Causality: k-tiles fully above the diagonal are skipped; diagonal
    tiles are zero-filled post-exp with gpsimd.affine_select.
  - V carries a ones column per head, so the PV matmul yields O^T plus the
    softmax denominators; O^T *= 1/den via DVE reciprocal + PE broadcast.
"""
import math

import numpy as np

import concourse.bass as bass
import concourse.tile as tile
from concourse import mybir, bacc
from concourse.bass_utils import run_bass_kernel_spmd

F32 = mybir.dt.float32
F32R = mybir.dt.float32r

B, T, C, H = 2, 2048, 1024, 16
D = C // H            # 64 head dim
NCORES = 8
HG = 4                # heads per core
CG = HG * D           # 256 channels per core
VW = HG * (D + 1)     # 260: V with a ones column per head
QB = 512              # q block width
KTW = 128             # k tile width
NQB = T // QB         # 4
NKT = T // KTW        # 16
NCH = C // 128        # 8 contraction chunks


def _slopes(n):
    def p2(m):
        start = 2 ** (-(2 ** -(math.log2(m) - 3)))
        return [start * start**i for i in range(m)]
    if math.log2(n).is_integer():
        return p2(n)
    c = 2 ** math.floor(math.log2(n))
    return p2(c) + _slopes(2 * c)[0::2][: n - c]


def _build():
    nc = bacc.Bacc()
    xt = nc.declare_dram_parameter("xt", [C + 1, T], F32R, isOutput=False)
    wq = nc.declare_dram_parameter("wq", [C + 1, CG], F32R, isOutput=False)
    wk = nc.declare_dram_parameter("wk", [C + 1, CG], F32R, isOutput=False)
    wv = nc.declare_dram_parameter("wv", [C + 1, VW], F32R, isOutput=False)
    wo = nc.declare_dram_parameter("wo", [CG, C], F32R, isOutput=False)
    hka = nc.declare_dram_parameter("hka", [HG, 2, T], F32R, isOutput=False)
    hqa = nc.declare_dram_parameter("hqa", [HG, 2, T], F32R, isOutput=False)
    y = nc.declare_dram_parameter("y", [T, C], F32, isOutput=True)

    EXP = mybir.ActivationFunctionType.Exp

    with tile.TileContext(nc) as tc, \
         nc.allow_low_precision(reason="fp32r compute"):
        with tc.tile_pool(name="const", bufs=1) as cp, \
             tc.tile_pool(name="xtp", bufs=14) as xtp, \
             tc.tile_pool(name="qap", bufs=8) as qap, \
             tc.tile_pool(name="otp", bufs=4) as otp, \
             tc.tile_pool(name="ptp", bufs=6) as ptp, \
             tc.tile_pool(name="yp", bufs=2) as ypool, \
             tc.tile_pool(name="misc", bufs=2) as mp, \
             tc.tile_pool(name="ps", bufs=6, space="PSUM") as psp, \
             tc.tile_pool(name="po", bufs=2, space="PSUM") as pop:

            # ---- constants: weights, aug rows, zero fill ----
            # DMA emission order matters for time-to-first-matmul: wq and
            # the first x block go first so the Q projection can start while
            # the rest of the constants stream in.
            wq_sb = [cp.tile([128, CG], F32R, tag=f"wq{c}", name=f"wq{c}") for c in range(NCH)]
            wk_sb = [cp.tile([128, CG], F32R, tag=f"wk{c}", name=f"wk{c}") for c in range(NCH)]
            wv_sb = [cp.tile([128, VW], F32R, tag=f"wv{c}", name=f"wv{c}") for c in range(NCH)]
            wo_sb = [cp.tile([128, C], F32R, tag=f"wo{c}", name=f"wo{c}") for c in range(2)]
            wqb = cp.tile([1, CG], F32R, tag="wqb")
            wkb = cp.tile([1, CG], F32R, tag="wkb")
            wvb = cp.tile([1, VW], F32R, tag="wvb")
            ones_sb = cp.tile([1, QB], F32R, tag="ones")
            xts0 = []
            for c in range(NCH):
                nc.sync.dma_start(wq_sb[c][:], wq[128 * c:128 * (c + 1), :])
                xtt = xtp.tile([128, QB], F32R, tag="xt", name=f"xt0_{c}")
                nc.sync.dma_start(xtt[:], xt[128 * c:128 * (c + 1), 0:QB])
                xts0.append(xtt)
            nc.sync.dma_start(wqb[:], wq[C:C + 1, :])
            nc.sync.dma_start(ones_sb[:], xt[C:C + 1, 0:QB])

            for c in range(NCH):
                nc.sync.dma_start(wk_sb[c][:], wk[128 * c:128 * (c + 1), :])
            nc.sync.dma_start(wkb[:], wk[C:C + 1, :])

            zf = cp.tile([128, QB], F32, tag="zf")
            nc.vector.memset(zf[:], 0.0)

            for c in range(NCH):
                nc.sync.dma_start(wv_sb[c][:], wv[128 * c:128 * (c + 1), :])
            nc.sync.dma_start(wvb[:], wv[C:C + 1, :])
            for c in range(2):
                nc.sync.dma_start(wo_sb[c][:], wo[128 * c:128 * (c + 1), :])

            # Per-head K^T tiles (resident, full T).  Even head: data rows
            # 0:64, aug rows 64:66, zeros 66:128.  Odd head: aug rows 0:2,
            # zeros 2:64, data rows 64:128.  K aug = (slope8*k, ones).
            ka = [cp.tile([128, T], F32R, tag=f"ka{h}", name=f"ka{h}") for h in range(HG)]
            for h in range(HG):
                par = h % 2
                arow = 64 if par == 0 else 0
                # zero the whole non-data half (32-aligned partition base),
                # then the aug-row DMA overwrites its 2 rows
                for blk in range(NQB):
                    sl = slice(QB * blk, QB * (blk + 1))
                    nc.vector.tensor_copy(ka[h][arow:arow + 64, sl],
                                          zf[arow:arow + 64, :])
                nc.sync.dma_start(ka[h][arow:arow + 2, :], hka[h])

            v_sb = [cp.tile([128, VW], F32R, tag=f"v{t}", name=f"v{t}") for t in range(NKT)]

            # ---- fused, software-pipelined per-block loop ----
            def proj(qb):
                """QKV projections for t-block qb; returns the Q tiles."""
                tsl = slice(QB * qb, QB * (qb + 1))
                if qb == 0:
                    xts = xts0
                else:
                    xts = []
                    for c in range(NCH):
                        xtt = xtp.tile([128, QB], F32R, tag="xt",
                                       name=f"xt{qb}_{c}")
                        nc.sync.dma_start(xtt[:],
                                          xt[128 * c:128 * (c + 1), tsl])
                        xts.append(xtt)

                qa_t = []
                for h in range(HG):
                    qat = qap.tile([128, QB], F32R, tag="qa",
                                   name=f"qa{qb}_{h}")
                    par = h % 2
                    arow = 64 if par == 0 else 0
                    nc.vector.tensor_copy(qat[arow:arow + 64, :],
                                          zf[arow:arow + 64, :])
                    nc.sync.dma_start(qat[arow:arow + 2, :], hqa[h][:, tsl])
                    qa_t.append(qat)

                for wsb, wb, is_q in ((wq_sb, wqb, True), (wk_sb, wkb, False)):
                    for m in range(2):
                        ps = psp.tile([128, QB], F32, tag="ps")
                        for c in range(NCH):
                            nc.tensor.matmul(
                                ps[:], wsb[c][:, 128 * m:128 * (m + 1)],
                                xts[c][:], start=(c == 0), stop=False,
                                skip_group_check=True)
                        nc.tensor.matmul(
                            ps[:], wb[:, 128 * m:128 * (m + 1)], ones_sb[:],
                            start=False, stop=True, skip_group_check=True)
                        for j in range(2):
                            h = 2 * m + j
                            rows = slice(64 * j, 64 * j + 64)
                            if is_q:
                                nc.vector.tensor_copy(qa_t[h][rows, :],
                                                      ps[rows, :])
                            else:
                                nc.vector.tensor_copy(ka[h][rows, tsl],
                                                      ps[rows, :])

                for tt in range(4):
                    kt = 4 * qb + tt
                    psv = psp.tile([128, QB], F32, tag="ps")
                    for c in range(NCH):
                        nc.tensor.matmul(
                            psv[:, 0:VW],
                            xts[c][:, 128 * tt:128 * (tt + 1)], wv_sb[c][:],
                            start=(c == 0), stop=False, skip_group_check=True)
                    nc.tensor.matmul(
                        psv[:, 0:VW], ones_sb[:, 0:128], wvb[:],
                        start=False, stop=True, skip_group_check=True)
                    nc.vector.tensor_copy(v_sb[kt][:], psv[:, 0:VW])
                return qa_t

            for qb in range(NQB):
                qa_t = proj(qb)
                # attention for this q-block.  Pass A per head is the
                # PE-heavy S/exp/mask/PV chain; pass B (recip -> broadcast
                # -> divide) for head h is emitted after head h+1's pass A
                # so the broadcast matmul never sits at the front of the PE
                # queue waiting on the DVE reciprocal.
                po_t = {}
                ot_t = [otp.tile([128, QB], F32R, tag="ot",
                                 name=f"ot_{qb}_{c}") for c in range(2)]

                def pass_a(h):
                    # diagonal tiles (which add a gpsimd mask hop to their
                    # exp chain) go first so the head's accumulation tail is
                    # short-latency and the reciprocal chain starts sooner
                    kts = list(range(4 * qb, 4 * qb + 4)) + list(range(4 * qb))
                    po = pop.tile([D + 1, QB], F32, tag="po",
                                  name=f"po_{qb}_{h}")
                    for i, kt in enumerate(kts):
                        pss = psp.tile([128, QB], F32, tag="ps")
                        nc.tensor.matmul(
                            pss[:], ka[h][:, 128 * kt:128 * (kt + 1)],
                            qa_t[h][:], start=True, stop=True,
                            skip_group_check=True)
                        pt = ptp.tile([128, QB], F32R, tag="pt")
                        nc.scalar.activation(pt[:], pss[:], EXP,
                                             bias=0.0, scale=0.125)
                        d0 = 128 * kt - QB * qb
                        if d0 >= 0:
                            # keep where k <= q, i.e. f - p - d0 >= 0
                            nc.gpsimd.affine_select(
                                pt[:], pt[:], pattern=[[1, QB]], base=-d0,
                                channel_multiplier=-1,
                                compare_op=mybir.AluOpType.is_ge, fill=0.0)
                        nc.tensor.matmul(
                            po[:], v_sb[kt][:, 65 * h:65 * (h + 1)], pt[:],
                            start=(i == 0), stop=(i == len(kts) - 1),
                            skip_group_check=True)
                    den = mp.tile([1, QB], F32, tag="den", bufs=2,
                                  name=f"den_{qb}_{h}")
                    nc.vector.tensor_copy(den[:], po[D:D + 1, :])
                    rc32 = mp.tile([1, QB], F32, tag="rc32", bufs=2,
                                   name=f"rc32_{qb}_{h}")
                    nc.vector.reciprocal_approx_fast(rc32[:], den[:])
                    rc = mp.tile([1, QB], F32R, tag="rc", bufs=4,
                                 name=f"rc_{qb}_{h}")
                    nc.vector.tensor_copy(rc[:], rc32[:])
                    po_t[h] = (po, rc)

                def pass_b(h):
                    po, rc = po_t.pop(h)
                    pb = psp.tile([D, QB], F32, tag="ps",
                                  name=f"pb_{qb}_{h}")
                    nc.tensor.matmul(pb[:], ones_sb[:, 0:D], rc[:],
                                     start=True, stop=True,
                                     skip_group_check=True)
                    bc = mp.tile([D, QB], F32, tag="bc", bufs=4,
                                 name=f"bc_{qb}_{h}")
                    nc.vector.tensor_copy(bc[:], pb[:])
                    pair = ot_t[h // 2]
                    if h % 2 == 0:
                        nc.vector.tensor_tensor(pair[0:D, :], po[0:D, :],
                                                bc[:],
                                                op=mybir.AluOpType.mult)
                    else:
                        # odd head's O^T lands at partitions 0:64; DVE
                        # cannot shift partitions, so divide into a temp
                        # then DMA it into rows 64:128 of the pair tile
                        tmp = mp.tile([D, QB], F32R, tag="ottmp", bufs=4,
                                      name=f"ottmp_{qb}_{h}")
                        nc.vector.tensor_tensor(tmp[:], po[0:D, :], bc[:],
                                                op=mybir.AluOpType.mult)
                        nc.sync.dma_start(pair[D:2 * D, :], tmp[:])

                for h in range(HG):
                    pass_a(h)
                    if h >= 1:
                        pass_b(h - 1)
                pass_b(HG - 1)

                # output projection for this t-block
                for tt in range(4):
                    t = 4 * qb + tt
                    fsl = slice(128 * tt, 128 * (tt + 1))
                    ysb = ypool.tile([128, C], F32, tag="y",
                                     name=f"y_{qb}_{tt}")
                    for half in range(2):
                        hsl = slice(QB * half, QB * (half + 1))
                        py = psp.tile([128, QB], F32, tag="ps")
                        for c in range(2):
                            nc.tensor.matmul(
                                py[:], ot_t[c][:, fsl], wo_sb[c][:, hsl],
                                start=(c == 0), stop=(c == 1),
                                skip_group_check=True)
                        nc.vector.tensor_copy(ysb[:, hsl], py[:])
                    nc.sync.dma_start(y[128 * t:128 * (t + 1), :], ysb[:])
    nc.finalize()
    return nc


_NC_CACHE = None


def _get_nc():
    global _NC_CACHE
    if _NC_CACHE is None:
        _NC_CACHE = _build()
    return _NC_CACHE


def kernel(x, Wq, bq, Wk, bk, Wv, bv, Wo, bo):
    x = np.asarray(x, dtype=np.float32)
    Wq, bq = np.asarray(Wq, np.float32), np.asarray(bq, np.float32)
    Wk, bk = np.asarray(Wk, np.float32), np.asarray(bk, np.float32)
    Wv, bv = np.asarray(Wv, np.float32), np.asarray(bv, np.float32)
    Wo, bo = np.asarray(Wo, np.float32), np.asarray(bo, np.float32)

    slopes = np.asarray(_slopes(H), dtype=np.float32)
    ar = np.arange(T, dtype=np.float32)

    xts = []
    for b in range(B):
        xa = np.empty((C + 1, T), np.float32)
        xa[:C] = x[b].T
        xa[C] = 1.0
        xts.append(np.ascontiguousarray(xa))

    shards = []
    for g in range(HG):
        csl = slice(CG * g, CG * (g + 1))
        wqa = np.concatenate([Wq[:, csl], bq[None, csl]], axis=0)
        wka = np.concatenate([Wk[:, csl], bk[None, csl]], axis=0)
        wva = np.zeros((C + 1, VW), np.float32)
        for j in range(HG):
            src = slice(CG * g + D * j, CG * g + D * (j + 1))
            wva[:C, 65 * j:65 * j + D] = Wv[:, src]
            wva[C, 65 * j:65 * j + D] = bv[src]
            wva[C, 65 * j + D] = 1.0
        woa = np.ascontiguousarray(Wo[csl, :])
        hk = np.empty((HG, 2, T), np.float32)
        hq = np.empty((HG, 2, T), np.float32)
        for j in range(HG):
            # K rows (k, s8) pair with Q rows (s8, -q): S += s8*(k - q).
            # Integer k/q are exact on the f32r grid and s8 rounds once, so
            # the large terms cancel exactly in the fp32 PSUM accumulator
            # (splitting s8*k / s8*q would round each entry independently
            # and leave O(s8*T*eps) noise in the scores).
            s8 = 8.0 * slopes[HG * g + j]
            hk[j, 0] = ar
            hk[j, 1] = s8
            hq[j, 0] = s8
            hq[j, 1] = -ar
        shards.append(dict(
            wq=np.ascontiguousarray(wqa), wk=np.ascontiguousarray(wka),
            wv=wva, wo=woa, hka=hk, hqa=hq))

    in_maps = []
    for core in range(NCORES):
        b, g = divmod(core, HG)
        m = dict(shards[g])
        m["xt"] = xts[b]
        in_maps.append(m)

    nc = _get_nc()
    res = run_bass_kernel_spmd(nc, in_maps, core_ids=list(range(NCORES)))

    out = np.empty((B, T, C), np.float32)
    for b in range(B):
        acc = res.results[4 * b]["y"].astype(np.float32).copy()
        for g in range(1, HG):
            acc += res.results[4 * b + g]["y"]
        out[b] = acc + bo[None, :]
    return out


# revision 38
# speedup vs baseline: 1.3656x; 1.2540x over previous
"""ALiBi causal attention layer on 8 TRN2 NeuronCores.

Sharding: data parallel on batch (B=2) x tensor parallel on heads (16 -> 4
groups of 4).  Core c = 4*b + g computes, for batch element b, the 4 heads
[4g, 4g+4) end to end: QKV projections (column-sharded), causal ALiBi
attention, and the row-sharded output projection.  The host sums the 4
partial outputs per batch element (the tensor-parallel all-reduce) and adds
the output bias.

Device kernel (all matmuls in float32r, ~1e-4 rel err, fp32 PSUM accum):
  - x arrives host-transposed with a ones row: xt [1025, 2048]; projection
    biases ride in an augmented contraction row of each weight matrix.
  - K^T lives in per-head [128, 2048] tiles: head data at its native
    partition parity (even head rows 0:64, odd rows 64:128), the ALiBi
    rank-2 rows (slope*8*k, ones) adjacent, remaining rows zeroed.  Q^T
    uses matching per-(head, q-block) [128, 512] tiles with rows
    (ones, -slope*8*q).  S^T = K_aug^T.T @ Q_aug then exp() directly on
    ACT with scale=1/8 (max-free softmax: scores are bounded), so S^T
    already includes the ALiBi bias.
  - Causality: k-tiles fully above the diagonal are skipped; diagonal
    tiles are zero-filled post-exp with gpsimd.affine_select.
  - V carries a ones column per head, so the PV matmul yields O^T plus the
    softmax denominators; O^T *= 1/den via DVE reciprocal + PE broadcast.
"""
import math

import numpy as np

import concourse.bass as bass
import concourse.tile as tile
from concourse import mybir, bacc
from concourse.bass_utils import run_bass_kernel_spmd

F32 = mybir.dt.float32
F32R = mybir.dt.float32r

B, T, C, H = 2, 2048, 1024, 16
D = C // H            # 64 head dim
NCORES = 8
HG = 4                # heads per core
CG = HG * D           # 256 channels per core
VW = HG * (D + 1)     # 260: V with a ones column per head
QB = 512              # q block width
KTW = 128             # k tile width
NQB = T // QB         # 4
NKT = T // KTW        # 16
NCH = C // 128        # 8 contraction chunks


def _slopes(n):
    def p2(m):
        start = 2 ** (-(2 ** -(math.log2(m) - 3)))
        return [start * start**i for i in range(m)]
    if math.log2(n).is_integer():
        return p2(n)
    c = 2 ** math.floor(math.log2(n))
    return p2(c) + _slopes(2 * c)[0::2][: n - c]


def _build():
    nc = bacc.Bacc()
    xt = nc.declare_dram_parameter("xt", [C + 1, T], F32R, isOutput=False)
    wq = nc.declare_dram_parameter("wq", [C + 1, CG], F32R, isOutput=False)
    wk = nc.declare_dram_parameter("wk", [C + 1, CG], F32R, isOutput=False)
    wv = nc.declare_dram_parameter("wv", [C + 1, VW], F32R, isOutput=False)
    wo = nc.declare_dram_parameter("wo", [CG, C], F32R, isOutput=False)
    hka = nc.declare_dram_parameter("hka", [HG, 2, T], F32R, isOutput=False)
    hqa = nc.declare_dram_parameter("hqa", [HG, 2, T], F32R, isOutput=False)
    y = nc.declare_dram_parameter("y", [T, C], F32, isOutput=True)

    EXP = mybir.ActivationFunctionType.Exp

    with tile.TileContext(nc) as tc, \
         nc.allow_low_precision(reason="fp32r compute"):
        with tc.tile_pool(name="const", bufs=1) as cp, \
             tc.tile_pool(name="xtp", bufs=14) as xtp, \
             tc.tile_pool(name="qap", bufs=8) as qap, \
             tc.tile_pool(name="otp", bufs=4) as otp, \
             tc.tile_pool(name="ptp", bufs=6) as ptp, \
             tc.tile_pool(name="yp", bufs=2) as ypool, \
             tc.tile_pool(name="misc", bufs=2) as mp, \
             tc.tile_pool(name="ps", bufs=6, space="PSUM") as psp, \
             tc.tile_pool(name="po", bufs=2, space="PSUM") as pop:

            # ---- constants: weights, aug rows, zero fill ----
            # DMA emission order matters for time-to-first-matmul: wq and
            # the first x block go first so the Q projection can start while
            # the rest of the constants stream in.
            wq_sb = [cp.tile([128, CG], F32R, tag=f"wq{c}", name=f"wq{c}") for c in range(NCH)]
            wk_sb = [cp.tile([128, CG], F32R, tag=f"wk{c}", name=f"wk{c}") for c in range(NCH)]
            wv_sb = [cp.tile([128, VW], F32R, tag=f"wv{c}", name=f"wv{c}") for c in range(NCH)]
            wo_sb = [cp.tile([128, C], F32R, tag=f"wo{c}", name=f"wo{c}") for c in range(2)]
            wqb = cp.tile([1, CG], F32R, tag="wqb")
            wkb = cp.tile([1, CG], F32R, tag="wkb")
            wvb = cp.tile([1, VW], F32R, tag="wvb")
            ones_sb = cp.tile([1, QB], F32R, tag="ones")
            xts0 = []
            for c in range(NCH):
                nc.sync.dma_start(wq_sb[c][:], wq[128 * c:128 * (c + 1), :])
                xtt = xtp.tile([128, QB], F32R, tag="xt", name=f"xt0_{c}")
                nc.sync.dma_start(xtt[:], xt[128 * c:128 * (c + 1), 0:QB])
                xts0.append(xtt)
            nc.sync.dma_start(wqb[:], wq[C:C + 1, :])
            nc.sync.dma_start(ones_sb[:], xt[C:C + 1, 0:QB])

            for c in range(NCH):
                nc.sync.dma_start(wk_sb[c][:], wk[128 * c:128 * (c + 1), :])
            nc.sync.dma_start(wkb[:], wk[C:C + 1, :])

            zf = cp.tile([128, QB], F32, tag="zf")
            nc.vector.memset(zf[:], 0.0)

            for c in range(NCH):
                nc.sync.dma_start(wv_sb[c][:], wv[128 * c:128 * (c + 1), :])
            nc.sync.dma_start(wvb[:], wv[C:C + 1, :])
            for c in range(2):
                nc.sync.dma_start(wo_sb[c][:], wo[128 * c:128 * (c + 1), :])

            # Per-head K^T tiles (resident, full T).  Even head: data rows
            # 0:64, aug rows 64:66, zeros 66:128.  Odd head: aug rows 0:2,
            # zeros 2:64, data rows 64:128.  K aug = (slope8*k, ones).
            ka = [cp.tile([128, T], F32R, tag=f"ka{h}", name=f"ka{h}") for h in range(HG)]
            for h in range(HG):
                par = h % 2
                arow = 64 if par == 0 else 0
                # zero the whole non-data half (32-aligned partition base),
                # then the aug-row DMA overwrites its 2 rows
                for blk in range(NQB):
                    sl = slice(QB * blk, QB * (blk + 1))
                    nc.vector.tensor_copy(ka[h][arow:arow + 64, sl],
                                          zf[arow:arow + 64, :])
                nc.sync.dma_start(ka[h][arow:arow + 2, :], hka[h])

            v_sb = [cp.tile([128, VW], F32R, tag=f"v{t}", name=f"v{t}") for t in range(NKT)]

            # ---- fused, software-pipelined per-block loop ----
            def proj(qb):
                """QKV projections for t-block qb; returns the Q tiles."""
                tsl = slice(QB * qb, QB * (qb + 1))
                if qb == 0:
                    xts = xts0
                else:
                    xts = []
                    for c in range(NCH):
                        xtt = xtp.tile([128, QB], F32R, tag="xt",
                                       name=f"xt{qb}_{c}")
                        nc.sync.dma_start(xtt[:],
                                          xt[128 * c:128 * (c + 1), tsl])
                        xts.append(xtt)

                qa_t = []
                for h in range(HG):
                    qat = qap.tile([128, QB], F32R, tag="qa",
                                   name=f"qa{qb}_{h}")
                    par = h % 2
                    arow = 64 if par == 0 else 0
                    nc.vector.tensor_copy(qat[arow:arow + 64, :],
                                          zf[arow:arow + 64, :])
                    nc.sync.dma_start(qat[arow:arow + 2, :], hqa[h][:, tsl])
                    qa_t.append(qat)

                for wsb, wb, is_q in ((wq_sb, wqb, True), (wk_sb, wkb, False)):
                    for m in range(2):
                        ps = psp.tile([128, QB], F32, tag="ps")
                        for c in range(NCH):
                            nc.tensor.matmul(
                                ps[:], wsb[c][:, 128 * m:128 * (m + 1)],
                                xts[c][:], start=(c == 0), stop=False,
                                skip_group_check=True)
                        nc.tensor.matmul(
                            ps[:], wb[:, 128 * m:128 * (m + 1)], ones_sb[:],
                            start=False, stop=True, skip_group_check=True)
                        for j in range(2):
                            h = 2 * m + j
                            rows = slice(64 * j, 64 * j + 64)
                            if is_q:
                                nc.vector.tensor_copy(qa_t[h][rows, :],
                                                      ps[rows, :])
                            else:
                                nc.vector.tensor_copy(ka[h][rows, tsl],
                                                      ps[rows, :])

                for tt in range(4):
                    kt = 4 * qb + tt
                    psv = psp.tile([128, QB], F32, tag="ps")
                    for c in range(NCH):
                        nc.tensor.matmul(
                            psv[:, 0:VW],
                            xts[c][:, 128 * tt:128 * (tt + 1)], wv_sb[c][:],
                            start=(c == 0), stop=False, skip_group_check=True)
                    nc.tensor.matmul(
                        psv[:, 0:VW], ones_sb[:, 0:128], wvb[:],
                        start=False, stop=True, skip_group_check=True)
                    nc.vector.tensor_copy(v_sb[kt][:], psv[:, 0:VW])
                return qa_t

            for qb in range(NQB):
                qa_t = proj(qb)
                # attention for this q-block.  Pass A per head is the
                # PE-heavy S/exp/mask/PV chain; pass B (recip -> broadcast
                # -> divide) for head h is emitted after head h+1's pass A
                # so the broadcast matmul never sits at the front of the PE
                # queue waiting on the DVE reciprocal.
                po_t = {}
                ot_t = [otp.tile([128, QB], F32R, tag="ot",
                                 name=f"ot_{qb}_{c}") for c in range(2)]

                # ALiBi windows per head slot: with the strided head
                # assignment, slot j holds global heads {4j..4j+3}; a tile
                # whose every (k, q) pair has slope*(k-q) <= -40 contributes
                # < 1e-16 to the softmax and is skipped outright.
                # W_j = 40 / min-slope-in-slot (slots 2, 3 keep everything).
                WIN = (160.0, 640.0, 1e9, 1e9)

                def pass_a(h):
                    # diagonal tiles (which add a gpsimd mask hop to their
                    # exp chain) go first so the head's accumulation tail is
                    # short-latency and the reciprocal chain starts sooner
                    full = [kt for kt in range(4 * qb)
                            if 128 * kt > QB * qb - WIN[h] - 127]
                    kts = list(range(4 * qb, 4 * qb + 4)) + full
                    po = pop.tile([D + 1, QB], F32, tag="po",
                                  name=f"po_{qb}_{h}")
                    for i, kt in enumerate(kts):
                        pss = psp.tile([128, QB], F32, tag="ps")
                        nc.tensor.matmul(
                            pss[:], ka[h][:, 128 * kt:128 * (kt + 1)],
                            qa_t[h][:], start=True, stop=True,
                            skip_group_check=True)
                        pt = ptp.tile([128, QB], F32R, tag="pt")
                        nc.scalar.activation(pt[:], pss[:], EXP,
                                             bias=0.0, scale=0.125)
                        d0 = 128 * kt - QB * qb
                        if d0 >= 0:
                            # keep where k <= q, i.e. f - p - d0 >= 0
                            nc.gpsimd.affine_select(
                                pt[:], pt[:], pattern=[[1, QB]], base=-d0,
                                channel_multiplier=-1,
                                compare_op=mybir.AluOpType.is_ge, fill=0.0)
                        nc.tensor.matmul(
                            po[:], v_sb[kt][:, 65 * h:65 * (h + 1)], pt[:],
                            start=(i == 0), stop=(i == len(kts) - 1),
                            skip_group_check=True)
                    den = mp.tile([1, QB], F32, tag="den", bufs=2,
                                  name=f"den_{qb}_{h}")
                    nc.vector.tensor_copy(den[:], po[D:D + 1, :])
                    rc32 = mp.tile([1, QB], F32, tag="rc32", bufs=2,
                                   name=f"rc32_{qb}_{h}")
                    nc.vector.reciprocal_approx_fast(rc32[:], den[:])
                    rc = mp.tile([1, QB], F32R, tag="rc", bufs=4,
                                 name=f"rc_{qb}_{h}")
                    nc.vector.tensor_copy(rc[:], rc32[:])
                    po_t[h] = (po, rc)

                def pass_b(h):
                    po, rc = po_t.pop(h)
                    pb = psp.tile([D, QB], F32, tag="ps",
                                  name=f"pb_{qb}_{h}")
                    nc.tensor.matmul(pb[:], ones_sb[:, 0:D], rc[:],
                                     start=True, stop=True,
                                     skip_group_check=True)
                    bc = mp.tile([D, QB], F32, tag="bc", bufs=4,
                                 name=f"bc_{qb}_{h}")
                    nc.vector.tensor_copy(bc[:], pb[:])
                    pair = ot_t[h // 2]
                    if h % 2 == 0:
                        nc.vector.tensor_tensor(pair[0:D, :], po[0:D, :],
                                                bc[:],
                                                op=mybir.AluOpType.mult)
                    else:
                        # odd head's O^T lands at partitions 0:64; DVE
                        # cannot shift partitions, so divide into a temp
                        # then DMA it into rows 64:128 of the pair tile
                        tmp = mp.tile([D, QB], F32R, tag="ottmp", bufs=4,
                                      name=f"ottmp_{qb}_{h}")
                        nc.vector.tensor_tensor(tmp[:], po[0:D, :], bc[:],
                                                op=mybir.AluOpType.mult)
                        nc.sync.dma_start(pair[D:2 * D, :], tmp[:])

                for h in range(HG):
                    pass_a(h)
                    if h >= 1:
                        pass_b(h - 1)
                pass_b(HG - 1)

                # output projection for this t-block
                for tt in range(4):
                    t = 4 * qb + tt
                    fsl = slice(128 * tt, 128 * (tt + 1))
                    ysb = ypool.tile([128, C], F32, tag="y",
                                     name=f"y_{qb}_{tt}")
                    for half in range(2):
                        hsl = slice(QB * half, QB * (half + 1))
                        py = psp.tile([128, QB], F32, tag="ps")
                        for c in range(2):
                            nc.tensor.matmul(
                                py[:], ot_t[c][:, fsl], wo_sb[c][:, hsl],
                                start=(c == 0), stop=(c == 1),
                                skip_group_check=True)
                        nc.vector.tensor_copy(ysb[:, hsl], py[:])
                    nc.sync.dma_start(y[128 * t:128 * (t + 1), :], ysb[:])
    nc.finalize()
    return nc


_NC_CACHE = None


def _get_nc():
    global _NC_CACHE
    if _NC_CACHE is None:
        _NC_CACHE = _build()
    return _NC_CACHE


def kernel(x, Wq, bq, Wk, bk, Wv, bv, Wo, bo):
    x = np.asarray(x, dtype=np.float32)
    Wq, bq = np.asarray(Wq, np.float32), np.asarray(bq, np.float32)
    Wk, bk = np.asarray(Wk, np.float32), np.asarray(bk, np.float32)
    Wv, bv = np.asarray(Wv, np.float32), np.asarray(bv, np.float32)
    Wo, bo = np.asarray(Wo, np.float32), np.asarray(bo, np.float32)

    slopes = np.asarray(_slopes(H), dtype=np.float32)
    ar = np.arange(T, dtype=np.float32)

    xts = []
    for b in range(B):
        xa = np.empty((C + 1, T), np.float32)
        xa[:C] = x[b].T
        xa[C] = 1.0
        xts.append(np.ascontiguousarray(xa))

    shards = []
    for g in range(HG):
        # strided head assignment: core g, slot j <-> global head 4j+g, so
        # each slot's ALiBi slope range is uniform across cores and the
        # (SPMD-shared) graph can window steep slots' attention
        heads = [HG * j + g for j in range(HG)]
        cols = np.concatenate([np.arange(D * h, D * (h + 1)) for h in heads])
        wqa = np.concatenate([Wq[:, cols], bq[None, cols]], axis=0)
        wka = np.concatenate([Wk[:, cols], bk[None, cols]], axis=0)
        wva = np.zeros((C + 1, VW), np.float32)
        for j, h in enumerate(heads):
            hsl = slice(D * h, D * (h + 1))
            wva[:C, 65 * j:65 * j + D] = Wv[:, hsl]
            wva[C, 65 * j:65 * j + D] = bv[hsl]
            wva[C, 65 * j + D] = 1.0
        woa = np.ascontiguousarray(Wo[cols, :])
        hk = np.empty((HG, 2, T), np.float32)
        hq = np.empty((HG, 2, T), np.float32)
        for j, h in enumerate(heads):
            # K rows (k, s8) pair with Q rows (s8, -q): S += s8*(k - q).
            # Integer k/q are exact on the f32r grid and s8 rounds once, so
            # the large terms cancel exactly in the fp32 PSUM accumulator
            # (splitting s8*k / s8*q would round each entry independently
            # and leave O(s8*T*eps) noise in the scores).
            s8 = 8.0 * slopes[h]
            hk[j, 0] = ar
            hk[j, 1] = s8
            hq[j, 0] = s8
            hq[j, 1] = -ar
        shards.append(dict(
            wq=np.ascontiguousarray(wqa), wk=np.ascontiguousarray(wka),
            wv=wva, wo=woa, hka=hk, hqa=hq))

    in_maps = []
    for core in range(NCORES):
        b, g = divmod(core, HG)
        m = dict(shards[g])
        m["xt"] = xts[b]
        in_maps.append(m)

    nc = _get_nc()
    res = run_bass_kernel_spmd(nc, in_maps, core_ids=list(range(NCORES)))

    out = np.empty((B, T, C), np.float32)
    for b in range(B):
        acc = res.results[4 * b]["y"].astype(np.float32).copy()
        for g in range(1, HG):
            acc += res.results[4 * b + g]["y"]
        out[b] = acc + bo[None, :]
    return out


# revision 39
# speedup vs baseline: 1.3662x; 1.0005x over previous
"""ALiBi causal attention layer on 8 TRN2 NeuronCores.

Sharding: data parallel on batch (B=2) x tensor parallel on heads (16 -> 4
groups of 4).  Core c = 4*b + g computes, for batch element b, the STRIDED
head set {g, 4+g, 8+g, 12+g} end to end: QKV projections (column-sharded),
causal ALiBi attention, and the row-sharded output projection.  The host
sums the 4 partial outputs per batch element (the tensor-parallel
all-reduce) and adds the output bias.  The striding makes head slot j hold
global heads {4j..4j+3} on every core, so each slot's ALiBi slope range is
uniform and the SPMD-shared graph can window steep slots' attention: slot 0
(slopes >= 0.25) looks back only 160 positions, slot 1 (>= 0.0625) 640 --
skipped k-tiles contribute < 1e-16 to the softmax.

Device kernel (all matmuls in float32r, ~1e-4 rel err, fp32 PSUM accum):
  - x arrives host-transposed with a ones row: xt [1025, 2048]; projection
    biases ride in an augmented contraction row of each weight matrix.
  - K^T lives in per-head [128, 2048] tiles: head data at its native
    partition parity (even head rows 0:64, odd rows 64:128), the ALiBi
    rank-2 rows (slope*8*k, ones) adjacent, remaining rows zeroed.  Q^T
    uses matching per-(head, q-block) [128, 512] tiles with rows
    (ones, -slope*8*q).  S^T = K_aug^T.T @ Q_aug then exp() directly on
    ACT with scale=1/8 (max-free softmax: scores are bounded), so S^T
    already includes the ALiBi bias.
  - Causality: k-tiles fully above the diagonal are skipped; diagonal
    tiles are zero-filled post-exp with gpsimd.affine_select.
  - V carries a ones column per head, so the PV matmul yields O^T plus the
    softmax denominators; O^T *= 1/den via DVE reciprocal + PE broadcast.
"""
import math

import numpy as np

import concourse.bass as bass
import concourse.tile as tile
from concourse import mybir, bacc
from concourse.bass_utils import run_bass_kernel_spmd

F32 = mybir.dt.float32
F32R = mybir.dt.float32r

B, T, C, H = 2, 2048, 1024, 16
D = C // H            # 64 head dim
NCORES = 8
HG = 4                # heads per core
CG = HG * D           # 256 channels per core
VW = HG * (D + 1)     # 260: V with a ones column per head
QB = 512              # q block width
KTW = 128             # k tile width
NQB = T // QB         # 4
NKT = T // KTW        # 16
NCH = C // 128        # 8 contraction chunks


def _slopes(n):
    def p2(m):
        start = 2 ** (-(2 ** -(math.log2(m) - 3)))
        return [start * start**i for i in range(m)]
    if math.log2(n).is_integer():
        return p2(n)
    c = 2 ** math.floor(math.log2(n))
    return p2(c) + _slopes(2 * c)[0::2][: n - c]


def _build():
    nc = bacc.Bacc()
    xt = nc.declare_dram_parameter("xt", [C + 1, T], F32R, isOutput=False)
    wq = nc.declare_dram_parameter("wq", [C + 1, CG], F32R, isOutput=False)
    wk = nc.declare_dram_parameter("wk", [C + 1, CG], F32R, isOutput=False)
    wv = nc.declare_dram_parameter("wv", [C + 1, VW], F32R, isOutput=False)
    wo = nc.declare_dram_parameter("wo", [CG, C], F32R, isOutput=False)
    hka = nc.declare_dram_parameter("hka", [HG, 2, T], F32R, isOutput=False)
    hqa = nc.declare_dram_parameter("hqa", [HG, 2, T], F32R, isOutput=False)
    y = nc.declare_dram_parameter("y", [T, C], F32, isOutput=True)

    EXP = mybir.ActivationFunctionType.Exp

    with tile.TileContext(nc) as tc, \
         nc.allow_low_precision(reason="fp32r compute"):
        with tc.tile_pool(name="const", bufs=1) as cp, \
             tc.tile_pool(name="xtp", bufs=14) as xtp, \
             tc.tile_pool(name="qap", bufs=8) as qap, \
             tc.tile_pool(name="otp", bufs=4) as otp, \
             tc.tile_pool(name="ptp", bufs=6) as ptp, \
             tc.tile_pool(name="yp", bufs=2) as ypool, \
             tc.tile_pool(name="misc", bufs=2) as mp, \
             tc.tile_pool(name="ps", bufs=6, space="PSUM") as psp, \
             tc.tile_pool(name="po", bufs=2, space="PSUM") as pop:

            # ---- constants: weights, aug rows, zero fill ----
            # DMA emission order matters for time-to-first-matmul: wq and
            # the first x block go first so the Q projection can start while
            # the rest of the constants stream in.
            wq_sb = [cp.tile([128, CG], F32R, tag=f"wq{c}", name=f"wq{c}") for c in range(NCH)]
            wk_sb = [cp.tile([128, CG], F32R, tag=f"wk{c}", name=f"wk{c}") for c in range(NCH)]
            wv_sb = [cp.tile([128, VW], F32R, tag=f"wv{c}", name=f"wv{c}") for c in range(NCH)]
            wo_sb = [cp.tile([128, C], F32R, tag=f"wo{c}", name=f"wo{c}") for c in range(2)]
            wqb = cp.tile([1, CG], F32R, tag="wqb")
            wkb = cp.tile([1, CG], F32R, tag="wkb")
            wvb = cp.tile([1, VW], F32R, tag="wvb")
            ones_sb = cp.tile([1, QB], F32R, tag="ones")
            xts0 = []
            for c in range(NCH):
                nc.sync.dma_start(wq_sb[c][:], wq[128 * c:128 * (c + 1), :])
                xtt = xtp.tile([128, QB], F32R, tag="xt", name=f"xt0_{c}")
                nc.sync.dma_start(xtt[:], xt[128 * c:128 * (c + 1), 0:QB])
                xts0.append(xtt)
            nc.sync.dma_start(wqb[:], wq[C:C + 1, :])
            nc.sync.dma_start(ones_sb[:], xt[C:C + 1, 0:QB])

            for c in range(NCH):
                nc.sync.dma_start(wk_sb[c][:], wk[128 * c:128 * (c + 1), :])
            nc.sync.dma_start(wkb[:], wk[C:C + 1, :])

            zf = cp.tile([128, QB], F32, tag="zf")
            nc.vector.memset(zf[:], 0.0)

            for c in range(NCH):
                nc.sync.dma_start(wv_sb[c][:], wv[128 * c:128 * (c + 1), :])
            nc.sync.dma_start(wvb[:], wv[C:C + 1, :])
            for c in range(2):
                nc.sync.dma_start(wo_sb[c][:], wo[128 * c:128 * (c + 1), :])

            # Per-head K^T tiles (resident, full T).  Even head: data rows
            # 0:64, aug rows 64:66, zeros 66:128.  Odd head: aug rows 0:2,
            # zeros 2:64, data rows 64:128.  K aug = (slope8*k, ones).
            ka = [cp.tile([128, T], F32R, tag=f"ka{h}", name=f"ka{h}") for h in range(HG)]
            for h in range(HG):
                par = h % 2
                arow = 64 if par == 0 else 0
                # zero the whole non-data half (32-aligned partition base),
                # then the aug-row DMA overwrites its 2 rows
                for blk in range(NQB):
                    sl = slice(QB * blk, QB * (blk + 1))
                    nc.vector.tensor_copy(ka[h][arow:arow + 64, sl],
                                          zf[arow:arow + 64, :])
                nc.sync.dma_start(ka[h][arow:arow + 2, :], hka[h])

            v_sb = [cp.tile([128, VW], F32R, tag=f"v{t}", name=f"v{t}") for t in range(NKT)]

            # ---- fused, software-pipelined per-block loop ----
            def proj(qb):
                """QKV projections for t-block qb; returns the Q tiles."""
                tsl = slice(QB * qb, QB * (qb + 1))
                if qb == 0:
                    xts = xts0
                else:
                    xts = []
                    for c in range(NCH):
                        xtt = xtp.tile([128, QB], F32R, tag="xt",
                                       name=f"xt{qb}_{c}")
                        nc.sync.dma_start(xtt[:],
                                          xt[128 * c:128 * (c + 1), tsl])
                        xts.append(xtt)

                qa_t = []
                for h in range(HG):
                    qat = qap.tile([128, QB], F32R, tag="qa",
                                   name=f"qa{qb}_{h}")
                    par = h % 2
                    arow = 64 if par == 0 else 0
                    nc.vector.tensor_copy(qat[arow:arow + 64, :],
                                          zf[arow:arow + 64, :])
                    nc.sync.dma_start(qat[arow:arow + 2, :], hqa[h][:, tsl])
                    qa_t.append(qat)

                for wsb, wb, is_q in ((wq_sb, wqb, True), (wk_sb, wkb, False)):
                    for m in range(2):
                        ps = psp.tile([128, QB], F32, tag="ps")
                        for c in range(NCH):
                            nc.tensor.matmul(
                                ps[:], wsb[c][:, 128 * m:128 * (m + 1)],
                                xts[c][:], start=(c == 0), stop=False,
                                skip_group_check=True)
                        nc.tensor.matmul(
                            ps[:], wb[:, 128 * m:128 * (m + 1)], ones_sb[:],
                            start=False, stop=True, skip_group_check=True)
                        for j in range(2):
                            h = 2 * m + j
                            rows = slice(64 * j, 64 * j + 64)
                            if is_q:
                                nc.vector.tensor_copy(qa_t[h][rows, :],
                                                      ps[rows, :])
                            else:
                                nc.vector.tensor_copy(ka[h][rows, tsl],
                                                      ps[rows, :])

                for tt in range(4):
                    kt = 4 * qb + tt
                    psv = psp.tile([128, QB], F32, tag="ps")
                    for c in range(NCH):
                        nc.tensor.matmul(
                            psv[:, 0:VW],
                            xts[c][:, 128 * tt:128 * (tt + 1)], wv_sb[c][:],
                            start=(c == 0), stop=False, skip_group_check=True)
                    nc.tensor.matmul(
                        psv[:, 0:VW], ones_sb[:, 0:128], wvb[:],
                        start=False, stop=True, skip_group_check=True)
                    nc.vector.tensor_copy(v_sb[kt][:], psv[:, 0:VW])
                return qa_t

            for qb in range(NQB):
                qa_t = proj(qb)
                # attention for this q-block.  Pass A per head is the
                # PE-heavy S/exp/mask/PV chain; pass B (recip -> broadcast
                # -> divide) for head h is emitted after head h+1's pass A
                # so the broadcast matmul never sits at the front of the PE
                # queue waiting on the DVE reciprocal.
                po_t = {}
                ot_t = [otp.tile([128, QB], F32R, tag="ot",
                                 name=f"ot_{qb}_{c}") for c in range(2)]

                # ALiBi windows per head slot: with the strided head
                # assignment, slot j holds global heads {4j..4j+3}; a tile
                # whose every (k, q) pair has slope*(k-q) <= -40 contributes
                # < 1e-16 to the softmax and is skipped outright.
                # W_j = 40 / min-slope-in-slot (slots 2, 3 keep everything).
                WIN = (160.0, 640.0, 1e9, 1e9)

                def pass_a(h):
                    # diagonal tiles (which add a gpsimd mask hop to their
                    # exp chain) go first so the head's accumulation tail is
                    # short-latency and the reciprocal chain starts sooner
                    full = [kt for kt in range(4 * qb)
                            if 128 * kt > QB * qb - WIN[h] - 127]
                    kts = list(range(4 * qb, 4 * qb + 4)) + full
                    po = pop.tile([D + 1, QB], F32, tag="po",
                                  name=f"po_{qb}_{h}")
                    for i, kt in enumerate(kts):
                        pss = psp.tile([128, QB], F32, tag="ps")
                        nc.tensor.matmul(
                            pss[:], ka[h][:, 128 * kt:128 * (kt + 1)],
                            qa_t[h][:], start=True, stop=True,
                            skip_group_check=True)
                        pt = ptp.tile([128, QB], F32R, tag="pt")
                        nc.scalar.activation(pt[:], pss[:], EXP,
                                             bias=0.0, scale=0.125)
                        d0 = 128 * kt - QB * qb
                        if d0 >= 0:
                            # keep where k <= q, i.e. f - p - d0 >= 0
                            nc.gpsimd.affine_select(
                                pt[:], pt[:], pattern=[[1, QB]], base=-d0,
                                channel_multiplier=-1,
                                compare_op=mybir.AluOpType.is_ge, fill=0.0)
                        nc.tensor.matmul(
                            po[:], v_sb[kt][:, 65 * h:65 * (h + 1)], pt[:],
                            start=(i == 0), stop=(i == len(kts) - 1),
                            skip_group_check=True)
                    den = mp.tile([1, QB], F32, tag="den", bufs=2,
                                  name=f"den_{qb}_{h}")
                    nc.vector.tensor_copy(den[:], po[D:D + 1, :])
                    rc32 = mp.tile([1, QB], F32, tag="rc32", bufs=2,
                                   name=f"rc32_{qb}_{h}")
                    nc.vector.reciprocal_approx_fast(rc32[:], den[:])
                    rc = mp.tile([1, QB], F32R, tag="rc", bufs=4,
                                 name=f"rc_{qb}_{h}")
                    nc.vector.tensor_copy(rc[:], rc32[:])
                    po_t[h] = (po, rc)

                def pass_b(h):
                    po, rc = po_t.pop(h)
                    pb = psp.tile([D, QB], F32, tag="ps",
                                  name=f"pb_{qb}_{h}")
                    nc.tensor.matmul(pb[:], ones_sb[:, 0:D], rc[:],
                                     start=True, stop=True,
                                     skip_group_check=True)
                    bc = mp.tile([D, QB], F32, tag="bc", bufs=4,
                                 name=f"bc_{qb}_{h}")
                    nc.vector.tensor_copy(bc[:], pb[:])
                    pair = ot_t[h // 2]
                    if h % 2 == 0:
                        nc.vector.tensor_tensor(pair[0:D, :], po[0:D, :],
                                                bc[:],
                                                op=mybir.AluOpType.mult)
                    else:
                        # odd head's O^T lands at partitions 0:64; DVE
                        # cannot shift partitions, so divide into a temp
                        # then DMA it into rows 64:128 of the pair tile
                        tmp = mp.tile([D, QB], F32R, tag="ottmp", bufs=4,
                                      name=f"ottmp_{qb}_{h}")
                        nc.vector.tensor_tensor(tmp[:], po[0:D, :], bc[:],
                                                op=mybir.AluOpType.mult)
                        nc.sync.dma_start(pair[D:2 * D, :], tmp[:])

                for h in range(HG):
                    pass_a(h)
                    if h >= 1:
                        pass_b(h - 1)
                pass_b(HG - 1)

                # output projection for this t-block
                for tt in range(4):
                    t = 4 * qb + tt
                    fsl = slice(128 * tt, 128 * (tt + 1))
                    ysb = ypool.tile([128, C], F32, tag="y",
                                     name=f"y_{qb}_{tt}")
                    for half in range(2):
                        hsl = slice(QB * half, QB * (half + 1))
                        py = psp.tile([128, QB], F32, tag="ps")
                        for c in range(2):
                            nc.tensor.matmul(
                                py[:], ot_t[c][:, fsl], wo_sb[c][:, hsl],
                                start=(c == 0), stop=(c == 1),
                                skip_group_check=True)
                        nc.vector.tensor_copy(ysb[:, hsl], py[:])
                    nc.sync.dma_start(y[128 * t:128 * (t + 1), :], ysb[:])
    nc.finalize()
    return nc


_NC_CACHE = None


def _get_nc():
    global _NC_CACHE
    if _NC_CACHE is None:
        _NC_CACHE = _build()
    return _NC_CACHE


def kernel(x, Wq, bq, Wk, bk, Wv, bv, Wo, bo):
    x = np.asarray(x, dtype=np.float32)
    Wq, bq = np.asarray(Wq, np.float32), np.asarray(bq, np.float32)
    Wk, bk = np.asarray(Wk, np.float32), np.asarray(bk, np.float32)
    Wv, bv = np.asarray(Wv, np.float32), np.asarray(bv, np.float32)
    Wo, bo = np.asarray(Wo, np.float32), np.asarray(bo, np.float32)

    slopes = np.asarray(_slopes(H), dtype=np.float32)
    ar = np.arange(T, dtype=np.float32)

    xts = []
    for b in range(B):
        xa = np.empty((C + 1, T), np.float32)
        xa[:C] = x[b].T
        xa[C] = 1.0
        xts.append(np.ascontiguousarray(xa))

    shards = []
    for g in range(HG):
        # strided head assignment: core g, slot j <-> global head 4j+g, so
        # each slot's ALiBi slope range is uniform across cores and the
        # (SPMD-shared) graph can window steep slots' attention
        heads = [HG * j + g for j in range(HG)]
        cols = np.concatenate([np.arange(D * h, D * (h + 1)) for h in heads])
        wqa = np.concatenate([Wq[:, cols], bq[None, cols]], axis=0)
        wka = np.concatenate([Wk[:, cols], bk[None, cols]], axis=0)
        wva = np.zeros((C + 1, VW), np.float32)
        for j, h in enumerate(heads):
            hsl = slice(D * h, D * (h + 1))
            wva[:C, 65 * j:65 * j + D] = Wv[:, hsl]
            wva[C, 65 * j:65 * j + D] = bv[hsl]
            wva[C, 65 * j + D] = 1.0
        woa = np.ascontiguousarray(Wo[cols, :])
        hk = np.empty((HG, 2, T), np.float32)
        hq = np.empty((HG, 2, T), np.float32)
        for j, h in enumerate(heads):
            # K rows (k, s8) pair with Q rows (s8, -q): S += s8*(k - q).
            # Integer k/q are exact on the f32r grid and s8 rounds once, so
            # the large terms cancel exactly in the fp32 PSUM accumulator
            # (splitting s8*k / s8*q would round each entry independently
            # and leave O(s8*T*eps) noise in the scores).
            s8 = 8.0 * slopes[h]
            hk[j, 0] = ar
            hk[j, 1] = s8
            hq[j, 0] = s8
            hq[j, 1] = -ar
        shards.append(dict(
            wq=np.ascontiguousarray(wqa), wk=np.ascontiguousarray(wka),
            wv=wva, wo=woa, hka=hk, hqa=hq))

    in_maps = []
    for core in range(NCORES):
        b, g = divmod(core, HG)
        m = dict(shards[g])
        m["xt"] = xts[b]
        in_maps.append(m)

    nc = _get_nc()
    res = run_bass_kernel_spmd(nc, in_maps, core_ids=list(range(NCORES)))

    out = np.empty((B, T, C), np.float32)
    for b in range(B):
        acc = res.results[4 * b]["y"].astype(np.float32).copy()
        for g in range(1, HG):
            acc += res.results[4 * b + g]["y"]
        out[b] = acc + bo[None, :]
    return out


# revision 40
# speedup vs baseline: 1.3922x; 1.0191x over previous
"""ALiBi causal attention layer on 8 TRN2 NeuronCores.

Sharding: data parallel on batch (B=2) x tensor parallel on heads (16 -> 4
groups of 4).  Core c = 4*b + g computes, for batch element b, the STRIDED
head set {g, 4+g, 8+g, 12+g} end to end: QKV projections (column-sharded),
causal ALiBi attention, and the row-sharded output projection.  The host
sums the 4 partial outputs per batch element (the tensor-parallel
all-reduce) and adds the output bias.  The striding makes head slot j hold
global heads {4j..4j+3} on every core, so each slot's ALiBi slope range is
uniform and the SPMD-shared graph can window steep slots' attention: slot 0
(slopes >= 0.25) looks back only 160 positions, slot 1 (>= 0.0625) 640 --
skipped k-tiles contribute < 1e-16 to the softmax.

Device kernel (all matmuls in float32r, ~1e-4 rel err, fp32 PSUM accum):
  - x arrives host-transposed with a ones row: xt [1025, 2048]; projection
    biases ride in an augmented contraction row of each weight matrix.
  - K^T lives in per-head [128, 2048] tiles: head data at its native
    partition parity (even head rows 0:64, odd rows 64:128), the ALiBi
    rank-2 rows (slope*8*k, ones) adjacent, remaining rows zeroed.  Q^T
    uses matching per-(head, q-block) [128, 512] tiles with rows
    (ones, -slope*8*q).  S^T = K_aug^T.T @ Q_aug then exp() directly on
    ACT with scale=1/8 (max-free softmax: scores are bounded), so S^T
    already includes the ALiBi bias.
  - Causality: k-tiles fully above the diagonal are skipped; diagonal
    tiles are zero-filled post-exp with gpsimd.affine_select.
  - V carries a ones column per head, so the PV matmul yields O^T plus the
    softmax denominators; O^T *= 1/den via DVE reciprocal + PE broadcast.
"""
import math

import numpy as np

import concourse.bass as bass
import concourse.tile as tile
from concourse import mybir, bacc
from concourse.bass_utils import run_bass_kernel_spmd

F32 = mybir.dt.float32
F32R = mybir.dt.float32r

B, T, C, H = 2, 2048, 1024, 16
D = C // H            # 64 head dim
NCORES = 8
HG = 4                # heads per core
CG = HG * D           # 256 channels per core
VW = HG * (D + 1)     # 260: V with a ones column per head
QB = 512              # q block width
KTW = 128             # k tile width
NQB = T // QB         # 4
NKT = T // KTW        # 16
NCH = C // 128        # 8 contraction chunks


def _slopes(n):
    def p2(m):
        start = 2 ** (-(2 ** -(math.log2(m) - 3)))
        return [start * start**i for i in range(m)]
    if math.log2(n).is_integer():
        return p2(n)
    c = 2 ** math.floor(math.log2(n))
    return p2(c) + _slopes(2 * c)[0::2][: n - c]


def _build():
    nc = bacc.Bacc()
    xt = nc.declare_dram_parameter("xt", [C + 1, T], F32R, isOutput=False)
    wq = nc.declare_dram_parameter("wq", [C + 1, CG], F32R, isOutput=False)
    wk = nc.declare_dram_parameter("wk", [C + 1, CG], F32R, isOutput=False)
    wv = nc.declare_dram_parameter("wv", [C + 1, VW], F32R, isOutput=False)
    wo = nc.declare_dram_parameter("wo", [CG, C], F32R, isOutput=False)
    hka = nc.declare_dram_parameter("hka", [HG, 2, T], F32R, isOutput=False)
    hqa = nc.declare_dram_parameter("hqa", [HG, 2, T], F32R, isOutput=False)
    y = nc.declare_dram_parameter("y", [T, C], F32, isOutput=True)

    EXP = mybir.ActivationFunctionType.Exp

    with tile.TileContext(nc) as tc, \
         nc.allow_low_precision(reason="fp32r compute"):
        with tc.tile_pool(name="const", bufs=1) as cp, \
             tc.tile_pool(name="xtp", bufs=14) as xtp, \
             tc.tile_pool(name="qap", bufs=8) as qap, \
             tc.tile_pool(name="otp", bufs=4) as otp, \
             tc.tile_pool(name="ptp", bufs=6) as ptp, \
             tc.tile_pool(name="yp", bufs=2) as ypool, \
             tc.tile_pool(name="misc", bufs=2) as mp, \
             tc.tile_pool(name="ps", bufs=6, space="PSUM") as psp, \
             tc.tile_pool(name="po", bufs=2, space="PSUM") as pop:

            # ---- constants: weights, aug rows, zero fill ----
            # DMA emission order matters for time-to-first-matmul: wq and
            # the first x block go first so the Q projection can start while
            # the rest of the constants stream in.
            wq_sb = [cp.tile([128, CG], F32R, tag=f"wq{c}", name=f"wq{c}") for c in range(NCH)]
            wk_sb = [cp.tile([128, CG], F32R, tag=f"wk{c}", name=f"wk{c}") for c in range(NCH)]
            wv_sb = [cp.tile([128, VW], F32R, tag=f"wv{c}", name=f"wv{c}") for c in range(NCH)]
            wo_sb = [cp.tile([128, C], F32R, tag=f"wo{c}", name=f"wo{c}") for c in range(2)]
            wqb = cp.tile([1, CG], F32R, tag="wqb")
            wkb = cp.tile([1, CG], F32R, tag="wkb")
            wvb = cp.tile([1, VW], F32R, tag="wvb")
            ones_sb = cp.tile([1, QB], F32R, tag="ones")
            xts0 = []
            for c in range(NCH):
                nc.sync.dma_start(wq_sb[c][:], wq[128 * c:128 * (c + 1), :])
                xtt = xtp.tile([128, QB], F32R, tag="xt", name=f"xt0_{c}")
                nc.sync.dma_start(xtt[:], xt[128 * c:128 * (c + 1), 0:QB])
                xts0.append(xtt)
            nc.sync.dma_start(wqb[:], wq[C:C + 1, :])
            nc.sync.dma_start(ones_sb[:], xt[C:C + 1, 0:QB])

            for c in range(NCH):
                nc.sync.dma_start(wk_sb[c][:], wk[128 * c:128 * (c + 1), :])
            nc.sync.dma_start(wkb[:], wk[C:C + 1, :])

            zf = cp.tile([128, QB], F32, tag="zf")
            nc.vector.memset(zf[:], 0.0)

            for c in range(NCH):
                nc.sync.dma_start(wv_sb[c][:], wv[128 * c:128 * (c + 1), :])
            nc.sync.dma_start(wvb[:], wv[C:C + 1, :])
            for c in range(2):
                nc.sync.dma_start(wo_sb[c][:], wo[128 * c:128 * (c + 1), :])

            # Per-head K^T tiles (resident, full T).  Even head: data rows
            # 0:64, aug rows 64:66, zeros 66:128.  Odd head: aug rows 0:2,
            # zeros 2:64, data rows 64:128.  K aug = (slope8*k, ones).
            ka = [cp.tile([128, T], F32R, tag=f"ka{h}", name=f"ka{h}") for h in range(HG)]
            for h in range(HG):
                par = h % 2
                arow = 64 if par == 0 else 0
                # zero the whole non-data half (32-aligned partition base),
                # then the aug-row DMA overwrites its 2 rows
                for blk in range(NQB):
                    sl = slice(QB * blk, QB * (blk + 1))
                    nc.vector.tensor_copy(ka[h][arow:arow + 64, sl],
                                          zf[arow:arow + 64, :])
                nc.sync.dma_start(ka[h][arow:arow + 2, :], hka[h])

            v_sb = [cp.tile([128, VW], F32R, tag=f"v{t}", name=f"v{t}") for t in range(NKT)]

            # ---- fused, software-pipelined per-block loop ----
            def proj(qb):
                """QKV projections for t-block qb; returns the Q tiles."""
                tsl = slice(QB * qb, QB * (qb + 1))
                if qb == 0:
                    xts = xts0
                else:
                    xts = []
                    for c in range(NCH):
                        xtt = xtp.tile([128, QB], F32R, tag="xt",
                                       name=f"xt{qb}_{c}")
                        nc.sync.dma_start(xtt[:],
                                          xt[128 * c:128 * (c + 1), tsl])
                        xts.append(xtt)

                qa_t = []
                for h in range(HG):
                    qat = qap.tile([128, QB], F32R, tag="qa",
                                   name=f"qa{qb}_{h}")
                    par = h % 2
                    arow = 64 if par == 0 else 0
                    nc.vector.tensor_copy(qat[arow:arow + 64, :],
                                          zf[arow:arow + 64, :])
                    nc.sync.dma_start(qat[arow:arow + 2, :], hqa[h][:, tsl])
                    qa_t.append(qat)

                for wsb, wb, is_q in ((wq_sb, wqb, True), (wk_sb, wkb, False)):
                    for m in range(2):
                        ps = psp.tile([128, QB], F32, tag="ps")
                        for c in range(NCH):
                            nc.tensor.matmul(
                                ps[:], wsb[c][:, 128 * m:128 * (m + 1)],
                                xts[c][:], start=(c == 0), stop=False,
                                skip_group_check=True)
                        nc.tensor.matmul(
                            ps[:], wb[:, 128 * m:128 * (m + 1)], ones_sb[:],
                            start=False, stop=True, skip_group_check=True)
                        for j in range(2):
                            h = 2 * m + j
                            rows = slice(64 * j, 64 * j + 64)
                            if is_q:
                                nc.vector.tensor_copy(qa_t[h][rows, :],
                                                      ps[rows, :])
                            else:
                                nc.vector.tensor_copy(ka[h][rows, tsl],
                                                      ps[rows, :])

                for tt in range(4):
                    kt = 4 * qb + tt
                    psv = psp.tile([128, QB], F32, tag="ps")
                    for c in range(NCH):
                        nc.tensor.matmul(
                            psv[:, 0:VW],
                            xts[c][:, 128 * tt:128 * (tt + 1)], wv_sb[c][:],
                            start=(c == 0), stop=False, skip_group_check=True)
                    nc.tensor.matmul(
                        psv[:, 0:VW], ones_sb[:, 0:128], wvb[:],
                        start=False, stop=True, skip_group_check=True)
                    nc.vector.tensor_copy(v_sb[kt][:], psv[:, 0:VW])
                return qa_t

            for qb in range(NQB):
                qa_t = proj(qb)
                # attention for this q-block.  Pass A per head is the
                # PE-heavy S/exp/mask/PV chain; pass B (recip -> broadcast
                # -> divide) for head h is emitted after head h+1's pass A
                # so the broadcast matmul never sits at the front of the PE
                # queue waiting on the DVE reciprocal.
                po_t = {}
                ot_t = [otp.tile([128, QB], F32R, tag="ot",
                                 name=f"ot_{qb}_{c}") for c in range(2)]

                # ALiBi windows per head slot: with the strided head
                # assignment, slot j holds global heads {4j..4j+3}; a tile
                # whose every (k, q) pair has slope*(k-q) <= -30 contributes
                # < 1e-11 to the softmax and is skipped outright.
                # W_j = 30 / min-slope-in-slot (slots 2, 3 keep everything).
                WIN = (120.0, 480.0, 1e9, 1e9)

                def pass_a(h):
                    # diagonal tiles (which add a gpsimd mask hop to their
                    # exp chain) go first so the head's accumulation tail is
                    # short-latency and the reciprocal chain starts sooner
                    full = [kt for kt in range(4 * qb)
                            if 128 * kt > QB * qb - WIN[h] - 127]
                    kts = list(range(4 * qb, 4 * qb + 4)) + full
                    po = pop.tile([D + 1, QB], F32, tag="po",
                                  name=f"po_{qb}_{h}")
                    for i, kt in enumerate(kts):
                        pss = psp.tile([128, QB], F32, tag="ps")
                        nc.tensor.matmul(
                            pss[:], ka[h][:, 128 * kt:128 * (kt + 1)],
                            qa_t[h][:], start=True, stop=True,
                            skip_group_check=True)
                        pt = ptp.tile([128, QB], F32R, tag="pt")
                        nc.scalar.activation(pt[:], pss[:], EXP,
                                             bias=0.0, scale=0.125)
                        d0 = 128 * kt - QB * qb
                        if d0 >= 0:
                            # keep where k <= q, i.e. f - p - d0 >= 0
                            nc.gpsimd.affine_select(
                                pt[:], pt[:], pattern=[[1, QB]], base=-d0,
                                channel_multiplier=-1,
                                compare_op=mybir.AluOpType.is_ge, fill=0.0)
                        nc.tensor.matmul(
                            po[:], v_sb[kt][:, 65 * h:65 * (h + 1)], pt[:],
                            start=(i == 0), stop=(i == len(kts) - 1),
                            skip_group_check=True)
                    den = mp.tile([1, QB], F32, tag="den", bufs=2,
                                  name=f"den_{qb}_{h}")
                    nc.vector.tensor_copy(den[:], po[D:D + 1, :])
                    rc32 = mp.tile([1, QB], F32, tag="rc32", bufs=2,
                                   name=f"rc32_{qb}_{h}")
                    nc.vector.reciprocal_approx_fast(rc32[:], den[:])
                    rc = mp.tile([1, QB], F32R, tag="rc", bufs=4,
                                 name=f"rc_{qb}_{h}")
                    nc.vector.tensor_copy(rc[:], rc32[:])
                    po_t[h] = (po, rc)

                def pass_b(h):
                    po, rc = po_t.pop(h)
                    pb = psp.tile([D, QB], F32, tag="ps",
                                  name=f"pb_{qb}_{h}")
                    nc.tensor.matmul(pb[:], ones_sb[:, 0:D], rc[:],
                                     start=True, stop=True,
                                     skip_group_check=True)
                    bc = mp.tile([D, QB], F32, tag="bc", bufs=4,
                                 name=f"bc_{qb}_{h}")
                    nc.vector.tensor_copy(bc[:], pb[:])
                    pair = ot_t[h // 2]
                    if h % 2 == 0:
                        nc.vector.tensor_tensor(pair[0:D, :], po[0:D, :],
                                                bc[:],
                                                op=mybir.AluOpType.mult)
                    else:
                        # odd head's O^T lands at partitions 0:64; DVE
                        # cannot shift partitions, so divide into a temp
                        # then DMA it into rows 64:128 of the pair tile
                        tmp = mp.tile([D, QB], F32R, tag="ottmp", bufs=4,
                                      name=f"ottmp_{qb}_{h}")
                        nc.vector.tensor_tensor(tmp[:], po[0:D, :], bc[:],
                                                op=mybir.AluOpType.mult)
                        nc.sync.dma_start(pair[D:2 * D, :], tmp[:])

                for h in range(HG):
                    pass_a(h)
                    if h >= 1:
                        pass_b(h - 1)
                pass_b(HG - 1)

                # output projection for this t-block
                for tt in range(4):
                    t = 4 * qb + tt
                    fsl = slice(128 * tt, 128 * (tt + 1))
                    ysb = ypool.tile([128, C], F32, tag="y",
                                     name=f"y_{qb}_{tt}")
                    for half in range(2):
                        hsl = slice(QB * half, QB * (half + 1))
                        py = psp.tile([128, QB], F32, tag="ps")
                        for c in range(2):
                            nc.tensor.matmul(
                                py[:], ot_t[c][:, fsl], wo_sb[c][:, hsl],
                                start=(c == 0), stop=(c == 1),
                                skip_group_check=True)
                        nc.vector.tensor_copy(ysb[:, hsl], py[:])
                    nc.sync.dma_start(y[128 * t:128 * (t + 1), :], ysb[:])
    nc.finalize()
    return nc


_NC_CACHE = None


def _get_nc():
    global _NC_CACHE
    if _NC_CACHE is None:
        _NC_CACHE = _build()
    return _NC_CACHE


def kernel(x, Wq, bq, Wk, bk, Wv, bv, Wo, bo):
    x = np.asarray(x, dtype=np.float32)
    Wq, bq = np.asarray(Wq, np.float32), np.asarray(bq, np.float32)
    Wk, bk = np.asarray(Wk, np.float32), np.asarray(bk, np.float32)
    Wv, bv = np.asarray(Wv, np.float32), np.asarray(bv, np.float32)
    Wo, bo = np.asarray(Wo, np.float32), np.asarray(bo, np.float32)

    slopes = np.asarray(_slopes(H), dtype=np.float32)
    ar = np.arange(T, dtype=np.float32)

    xts = []
    for b in range(B):
        xa = np.empty((C + 1, T), np.float32)
        xa[:C] = x[b].T
        xa[C] = 1.0
        xts.append(np.ascontiguousarray(xa))

    shards = []
    for g in range(HG):
        # strided head assignment: core g, slot j <-> global head 4j+g, so
        # each slot's ALiBi slope range is uniform across cores and the
        # (SPMD-shared) graph can window steep slots' attention
        heads = [HG * j + g for j in range(HG)]
        cols = np.concatenate([np.arange(D * h, D * (h + 1)) for h in heads])
        wqa = np.concatenate([Wq[:, cols], bq[None, cols]], axis=0)
        wka = np.concatenate([Wk[:, cols], bk[None, cols]], axis=0)
        wva = np.zeros((C + 1, VW), np.float32)
        for j, h in enumerate(heads):
            hsl = slice(D * h, D * (h + 1))
            wva[:C, 65 * j:65 * j + D] = Wv[:, hsl]
            wva[C, 65 * j:65 * j + D] = bv[hsl]
            wva[C, 65 * j + D] = 1.0
        woa = np.ascontiguousarray(Wo[cols, :])
        hk = np.empty((HG, 2, T), np.float32)
        hq = np.empty((HG, 2, T), np.float32)
        for j, h in enumerate(heads):
            # K rows (k, s8) pair with Q rows (s8, -q): S += s8*(k - q).
            # Integer k/q are exact on the f32r grid and s8 rounds once, so
            # the large terms cancel exactly in the fp32 PSUM accumulator
            # (splitting s8*k / s8*q would round each entry independently
            # and leave O(s8*T*eps) noise in the scores).
            s8 = 8.0 * slopes[h]
            hk[j, 0] = ar
            hk[j, 1] = s8
            hq[j, 0] = s8
            hq[j, 1] = -ar
        shards.append(dict(
            wq=np.ascontiguousarray(wqa), wk=np.ascontiguousarray(wka),
            wv=wva, wo=woa, hka=hk, hqa=hq))

    in_maps = []
    for core in range(NCORES):
        b, g = divmod(core, HG)
        m = dict(shards[g])
        m["xt"] = xts[b]
        in_maps.append(m)

    nc = _get_nc()
    res = run_bass_kernel_spmd(nc, in_maps, core_ids=list(range(NCORES)))

    out = np.empty((B, T, C), np.float32)
    for b in range(B):
        acc = res.results[4 * b]["y"].astype(np.float32).copy()
        for g in range(1, HG):
            acc += res.results[4 * b + g]["y"]
        out[b] = acc + bo[None, :]
    return out
